# revision 39
# baseline (speedup 1.0000x reference)
"""FFM layer (nn_FFM_Layer) Trainium2 Bass kernel — 2048B quantized-row gather.

Reference computation (B=4096, 13 dense fields, 26 sparse fields with vocab
1000 each, FIELD_NUM=39, K=16):

    idx        = sparse + offsets                      # [B, 26] global ids
    first      = w0 + dense @ w[:13] + sum_j w[idx]    # [B, 1]
    field_f    = einsum('bd,dfk', dense, v[:13]) + sum_j v[idx]   # [B,39,16]
    s          = field_f.sum(1)                        # [B, 16]
    second     = 0.5*(||s||^2 - sum_fk field_f^2)      # [B]
    out        = first + second[:, None]

Strategy (data-parallel over batch, 8 cores x 512 samples, no collectives).
The f32 baseline (2560B gathered rows) ran at the 16-DMA-engine wall
(~390 GB/s/core): 34.1 MB of gather -> ~98us stream, 128.4us total.  This
version cuts gathered bytes 20% with a quantized row while keeping f32-level
absolute accuracy (the output has near-zero entries; the max-rel-err metric
tolerates only ~2e-5 absolute error):

  Row [2048 B] = [624 fp16 v | 624 int8 resid | 16 f32 Vs | 1 f32 w | pad]
    - fp16 v plus int8 residual (scale 2^-20) gives ~21 effective mantissa
      bits per entry (per-entry err ~2.8e-7).
    - Vs[i] = sum_f v[i,f,:] (16 f32, computed in f64 host-side) makes the
      field-sum s EXACT; without it the FM-identity cancellation amplifies
      the fp16 noise ~40x (host-measured 7e-2 max rel vs 4.4e-3 with it).
    - w col has w0 folded into sparse-table-0 rows (hit exactly once).

  Engine split per 128-sample chunk (26 gathered rows each):
    - PE: dense seed matmuls (f32) + 26 identity-matmul accumulations of the
      fp16 sections into f32 PSUM (exact: fp16->f32 widening + f32 adds).
    - ACT: converts each call's int8 residual block to fp16 (exact for
      |r|<=127); ACT is otherwise idle.
    - DVE: folds the fp16 residual cols (2x 16-bit mode; integer sums <=3302
      round at most ~1 ulp = 2^-20 * 1), folds the 17 f32 Vs/w cols, and
      runs the WHOLE FM epilogue: combine, squares via scalar_tensor_tensor
      with accum_out, and the final scale-bias.  Keeping ACT convert-only is
      load-bearing: an ACT epilogue op waiting on the combine head-of-line
      blocks the next chunk's converts in ACT's in-order queue (~5.5us/chunk,
      measured; moving the epilogue to DVE took 127 -> 113us).

  Gather calls are sample-chunk-major (8 calls of 4/2 fields x 128 samples)
  with even field counts per call, alternating across 2 SWDGE queues
  (desc-gen backpressure tracks the DMA bus; single_packet=False measured
  ~7us faster end-to-end; >=1536 descriptors per call wedges the exec unit).
  Call-granularity scan (HW): 8-field calls 117us, 6-field 112us, the
  [4x5,2x3] taper 107.9us, 10 calls of mostly-2 117us -- finer calls fill
  the SWDGE queues sooner (faster ramp) and shorten the fold tail, until
  per-call fixed overhead (~1us desc-gen startup) dominates.  4 SWDGE
  queues measured ~30us slower than 2.

  FGROUPS_FIRST leads with 2-field calls ([4,2,2,2,...]) purely to fill
  both SWDGE queues sooner -- measured ~1us faster ramp.

  Measured: 107.1us best / ~107.5 typ (vs 128.4us f32 baseline): ~16.5us
  prologue (entry barriers + ucode lib load + ~8us fixed HWDGE latency on
  the idx load), ~78us gather stream at ~99% of the 16-DMA-engine wall
  (22.9 B/ns/engine), ~7us tail (last call convert+fold+epilogue chain +
  final drain/barrier).  Device-level fast/slow drift (~108 vs ~120-127us
  for identical code, shared-chip DMA contention) dominates residual
  variance.
"""

import sys

if "/opt/trn_rl_repo" not in sys.path:
    sys.path.insert(0, "/opt/trn_rl_repo")

import numpy as np

import concourse.bacc as bacc
import concourse.bass as bass
import concourse.tile as tile
from concourse import mybir
from concourse.bass_utils import run_bass_kernel_spmd

# Problem constants (hardcoded per harness contract)
B = 4096
N_DENSE = 13
N_SPARSE = 26
FEAT_PER_SPARSE = 1000
FIELD_NUM = 39
FEATURE_NUM = 26013
K = 16
N_CORES = 8
BC = B // N_CORES          # 512 samples per core
VCOLS = FIELD_NUM * K      # 624
P = 128
SCHUNKS = BC // P          # 4 sample chunks of 128 per core

# --- quantized row layout (bytes) ---
ROWB = 2048                # gathered row: mult of 256 required by dma_gather
OFF16 = 0                  # 624 fp16          [0, 1248)
OFF8 = 1248                # 624 int8 resid    [1248, 1872)
OFFV = 1872                # 16 f32 Vs         [1872, 1936)
OFFW = 1936                # 1 f32 w           [1936, 1940)
RSCALE = float(2.0 ** -20)

# per-chunk gather calls: even field groups (sum 26), each call nf*128 idxs.
# First group per chunk has nf>=4 so the fold accumulators init via pair-add
# (tensor_copy measured ~1.8us vs ~0.5us for the add path).
FGROUPS = [4, 4, 4, 4, 4, 2, 2, 2]
FGROUPS_FIRST = [4, 2, 2, 2, 2, 2, 4, 4, 4]
FGROUPS_LAST = [4, 4, 4, 4, 4, 2, 2, 2]
IDX_COLS_SC = N_SPARSE * P // 16   # 208 idx cols per sample chunk

F32 = mybir.dt.float32
F16 = mybir.dt.float16
I16 = mybir.dt.int16
I8 = mybir.dt.int8
U8 = mybir.dt.uint8


def build_program():
    """Build + compile the single-core SPMD bass program."""
    nc = bacc.Bacc("TRN2", target_bir_lowering=False, debug=False,
                   num_swdge_queues=2)

    vaug_t = nc.dram_tensor("vaug", [FEATURE_NUM, ROWB], U8, kind="ExternalInput")
    # dense rhs [13, 641] = [v[:13] 624 | Vs[:13] 16 | w[:13] 1]
    v13_t = nc.dram_tensor("vaug13f", [N_DENSE, VCOLS + K + 1], F32,
                           kind="ExternalInput")
    dense_t = nc.dram_tensor("dense_t", [N_DENSE, BC], F32, kind="ExternalInput")
    idxs_t = nc.dram_tensor("idxs", [P, SCHUNKS * IDX_COLS_SC], I16,
                            kind="ExternalInput")
    ident_t = nc.dram_tensor("ident16", [P, P], F16, kind="ExternalInput")
    out_t = nc.dram_tensor("out", [P, SCHUNKS], F32, kind="ExternalOutput")

    with tile.TileContext(nc) as tc:
        with (
            tc.tile_pool(name="main", bufs=1) as main,
            tc.tile_pool(name="gath", bufs=9) as gath,
            tc.tile_pool(name="cvt", bufs=5) as cvt,
            tc.tile_pool(name="sqp", bufs=2) as sqp,
            tc.tile_pool(name="fold", bufs=2) as fold,
            tc.tile_pool(name="small", bufs=2) as small,
            tc.tile_pool(name="psum", bufs=4, space="PSUM") as psum,
        ):
            # idx chunk 0 first so the first gather can start ASAP
            idx_sbs = []
            for c in range(SCHUNKS):
                t = main.tile([P, IDX_COLS_SC], I16, tag=f"idx{c}")
                nc.sync.dma_start(
                    t[:], idxs_t[:, c * IDX_COLS_SC : (c + 1) * IDX_COLS_SC]
                )
                idx_sbs.append(t)
            # remaining inputs on the scalar engine's DMA queue
            v13 = main.tile([N_DENSE, VCOLS + K + 1], F32)
            nc.scalar.dma_start(v13[:], v13_t[:])
            dt_sb = main.tile([N_DENSE, BC], F32)
            nc.scalar.dma_start(dt_sb[:], dense_t[:])
            ident = main.tile([P, P], F16)
            nc.scalar.dma_start(ident[:], ident_t[:])

            res = main.tile([P, SCHUNKS], F32)

            def epilogue(c, ps, racc, accv):
                """FM identity epilogue for chunk c — entirely on DVE+sync.
                Keeping ACT convert-only matters: an ACT epilogue op waiting
                on the combine head-of-line blocks the next chunk's converts
                in ACT's in-order queue (measured ~5.5us/chunk)."""
                # fld = PSUM(fp16 sums + dense) + 2^-20 * resid_sum
                fld = fold.tile([P, VCOLS], F32, tag="fld")
                nc.vector.scalar_tensor_tensor(
                    out=fld[:], in0=racc[:], scalar=RSCALE,
                    in1=ps[:, 0:VCOLS],
                    op0=mybir.AluOpType.mult, op1=mybir.AluOpType.add,
                )
                # s cols + w col in one PSUM-reading add
                stw = small.tile([P, K + 1], F32, tag="stw")
                nc.vector.tensor_add(stw[:], ps[:, VCOLS:VCOLS + K + 1],
                                     accv[:])
                # q = ||fld||^2, snorm = ||s||^2 via DVE squares with accum
                sq = sqp.tile([P, VCOLS], F32, tag="sq")
                q = small.tile([P, 1], F32, tag="q")
                nc.vector.scalar_tensor_tensor(
                    out=sq[:], in0=fld[:], scalar=1.0, in1=fld[:],
                    op0=mybir.AluOpType.mult, op1=mybir.AluOpType.mult,
                    accum_out=q[:],
                )
                s2 = small.tile([P, K], F32, tag="s2")
                snorm = small.tile([P, 1], F32, tag="snorm")
                nc.vector.scalar_tensor_tensor(
                    out=s2[:], in0=stw[:, 0:K], scalar=1.0, in1=stw[:, 0:K],
                    op0=mybir.AluOpType.mult, op1=mybir.AluOpType.mult,
                    accum_out=snorm[:],
                )
                diff = small.tile([P, 1], F32, tag="diff")
                nc.vector.tensor_tensor(
                    out=diff[:], in0=snorm[:], in1=q[:],
                    op=mybir.AluOpType.subtract,
                )
                # out = 0.5*diff + (w-sum incl. w0 and dense first-order)
                nc.vector.scalar_tensor_tensor(
                    out=res[:, c : c + 1], in0=diff[:], scalar=0.5,
                    in1=stw[:, K:K + 1],
                    op0=mybir.AluOpType.mult, op1=mybir.AluOpType.add,
                )
                nc.sync.dma_start(out_t[:, c : c + 1], res[:, c : c + 1])

            call_no = 0
            pending = None  # (c, ps, racc, accv) awaiting epilogue
            for c in range(SCHUNKS):
                # dense part seeds the three PSUM chains:
                #   A: fld[0:512]   B: fld[512:624]+s[624:640]... C: [624:641]
                ps = psum.tile([P, VCOLS + K + 1], F32, tag="ps")
                lhs_d = dt_sb[:, c * P : (c + 1) * P]
                nc.tensor.matmul(out=ps[:, 0:512], lhsT=lhs_d,
                                 rhs=v13[:, 0:512], start=True, stop=False)
                nc.tensor.matmul(out=ps[:, 512:VCOLS + K + 1], lhsT=lhs_d,
                                 rhs=v13[:, 512:VCOLS + K + 1],
                                 start=True, stop=False)

                if c == 0:
                    fgroups = FGROUPS_FIRST
                elif c == SCHUNKS - 1:
                    fgroups = FGROUPS_LAST
                else:
                    fgroups = FGROUPS

                racc = fold.tile([P, VCOLS], F16, tag="racc")
                accv = small.tile([P, K + 1], F32, tag="accv")
                icol = 0
                for gi, nf in enumerate(fgroups):
                    last_call = gi == len(fgroups) - 1
                    n_idx = nf * P
                    g = gath.tile([P, 6, ROWB], U8, tag="g")
                    nc.gpsimd.dma_gather(
                        g[:, :nf, :],
                        vaug_t[:],
                        idx_sbs[c][:, icol : icol + n_idx // 16],
                        n_idx,
                        n_idx,
                        ROWB,
                        single_packet=False,
                        queue_num=call_no % 2,
                    )
                    icol += n_idx // 16
                    call_no += 1

                    # PE: accumulate the fp16 sections into PSUM chains A/B
                    g16 = g[:, :nf, OFF16 : OFF16 + 2 * VCOLS].bitcast(F16)
                    for j in range(nf):
                        stop = last_call and j == nf - 1
                        nc.tensor.matmul(out=ps[:, 0:512], lhsT=ident[:],
                                         rhs=g16[:, j, 0:512],
                                         start=False, stop=stop)
                        nc.tensor.matmul(out=ps[:, 512:VCOLS], lhsT=ident[:],
                                         rhs=g16[:, j, 512:VCOLS],
                                         start=False, stop=stop)

                    # ACT: exact int8 -> fp16 convert of the residual block
                    g8 = g[:, :nf, OFF8 : OFF8 + VCOLS].bitcast(I8)
                    r16 = cvt.tile([P, 6, VCOLS], F16, tag="r16")
                    nc.scalar.activation(
                        r16[:, :nf, :], g8,
                        mybir.ActivationFunctionType.Identity,
                    )
                    # DVE: fold residual (2x fp16), fp16 tail cols (f32 out,
                    # exact), and the 17 f32 Vs/w cols
                    gv = g[:, :nf, OFFV : OFFV + 4 * (K + 1)].bitcast(F32)
                    hp = nf // 2
                    pr = fold.tile([P, 3, VCOLS], F16, tag="pr")
                    nc.vector.tensor_add(pr[:, :hp, :],
                                         r16[:, 0:nf:2, :],
                                         r16[:, 1:nf:2, :])
                    pv = small.tile([P, 3, K + 1], F32, tag="pv")
                    nc.vector.tensor_add(pv[:, :hp, :],
                                         gv[:, 0:nf:2, :],
                                         gv[:, 1:nf:2, :])
                    h0 = 0
                    if gi == 0:
                        # first call of the chunk: init accumulators by add
                        nc.vector.tensor_add(racc[:], pr[:, 0, :], pr[:, 1, :])
                        nc.vector.tensor_add(accv[:], pv[:, 0, :], pv[:, 1, :])
                        h0 = 2
                    for h in range(h0, hp):
                        nc.vector.tensor_add(racc[:], racc[:], pr[:, h, :])
                        nc.vector.tensor_add(accv[:], accv[:], pv[:, h, :])

                    if gi == 0 and pending is not None:
                        # previous chunk's epilogue, behind this chunk's first
                        # convert in the ACT queue
                        epilogue(*pending)
                        pending = None

                pending = (c, ps, racc, accv)

            epilogue(*pending)
            pending = None

    nc.compile()
    return nc


def prep_inputs(dense_inputs, sparse_inputs, w0, w, v):
    """Host-side shard/pack: build per-core in_maps."""
    dense = np.asarray(dense_inputs, np.float32)
    sparse = np.asarray(sparse_inputs)
    w0 = np.asarray(w0, np.float32)
    w = np.asarray(w, np.float32)
    v = np.asarray(v, np.float32).reshape(FEATURE_NUM, VCOLS)

    v16 = v.astype(np.float16)
    delta = v - v16.astype(np.float32)
    resid = np.clip(np.round(delta * (1.0 / RSCALE)), -127, 127).astype(np.int8)
    vs = (v.reshape(FEATURE_NUM, FIELD_NUM, K)
           .sum(axis=1, dtype=np.float64).astype(np.float32))
    wcol = w[:, 0].copy()
    # fold w0 into sparse table 0 (each sample hits it exactly once)
    wcol[N_DENSE : N_DENSE + FEAT_PER_SPARSE] += w0[0]

    vaug = np.zeros((FEATURE_NUM, ROWB), np.uint8)
    vaug[:, OFF16 : OFF16 + 2 * VCOLS] = v16.view(np.uint8)
    vaug[:, OFF8 : OFF8 + VCOLS] = resid.view(np.uint8)
    vaug[:, OFFV : OFFV + 4 * K] = np.ascontiguousarray(vs).view(np.uint8)
    vaug[:, OFFW : OFFW + 4] = np.ascontiguousarray(
        wcol[:, None]).view(np.uint8)

    v13f = np.zeros((N_DENSE, VCOLS + K + 1), np.float32)
    v13f[:, :VCOLS] = v[:N_DENSE]
    v13f[:, VCOLS : VCOLS + K] = vs[:N_DENSE]
    v13f[:, VCOLS + K] = w[:N_DENSE, 0]

    offs = N_DENSE + FEAT_PER_SPARSE * np.arange(N_SPARSE, dtype=np.int64)
    gidx = (sparse.astype(np.int64) + offs[None, :]).astype(np.int16)  # [B, 26]

    ident16 = np.eye(P, dtype=np.float16)

    in_maps = []
    for core in range(N_CORES):
        sl = slice(core * BC, (core + 1) * BC)
        dt = np.ascontiguousarray(dense[sl].T)          # [13, 512]
        idxc = gidx[sl]                                 # [512, 26]
        buf = np.zeros((P, SCHUNKS * IDX_COLS_SC), np.int16)
        off_c = 0
        for c in range(SCHUNKS):
            rows = idxc[c * P : (c + 1) * P]            # [128, 26]
            fbase = 0
            for nf in (FGROUPS_FIRST if c == 0 else
                       (FGROUPS_LAST if c == SCHUNKS - 1 else FGROUPS)):
                n = nf * P
                # call order: i = f_local*128 + p  ->  row idx[p, fbase+f]
                seg = np.ascontiguousarray(
                    rows[:, fbase : fbase + nf].T
                ).reshape(-1)                           # [nf*128]
                wrapped = seg.reshape(n // 16, 16).T    # [16, n/16]
                buf[:, off_c : off_c + n // 16] = np.tile(wrapped, (8, 1))
                fbase += nf
                off_c += n // 16
        in_maps.append({"vaug": vaug, "vaug13f": v13f, "dense_t": dt,
                        "idxs": buf, "ident16": ident16})
    return in_maps


_NC_CACHE = None


def kernel(dense_inputs, sparse_inputs, w0, w, v):
    global _NC_CACHE
    if _NC_CACHE is None:
        _NC_CACHE = build_program()
    nc = _NC_CACHE
    in_maps = prep_inputs(dense_inputs, sparse_inputs, w0, w, v)
    res = run_bass_kernel_spmd(nc, in_maps, core_ids=list(range(N_CORES)))
    outs = []
    for r in res.results:
        o = r["out"]                                    # [128, 4]
        outs.append(np.ascontiguousarray(o.T).reshape(BC, 1))
    return np.concatenate(outs, axis=0).astype(np.float32)
